# revision 12
# baseline (speedup 1.0000x reference)
# Trainium2 Bass kernel for nn_AttentionCombiner — collective-free.
#
# Sharding: 8 cores = 4 batches x 2 q-halves (all 8 heads per core).
# The host reorders the keys per core so l-blocks 0..7 are the core's OWN
# q-rows and 8..15 the peer's (identity for even cores). Softmax row-sums
# are then computed fully locally:
#   - own half  (Sum_{l in my half} S[l,q] = Sum_{q'} S[q,q'] by symmetry):
#     DVE tensor_reduce over the exp'd S tiles (ACT accum_out for one
#     tile/head to keep DVE inside its per-head window);
#   - peer half: column sums of S tiles 8..15 via ap_size-1 ones-matmuls on
#     PE (8 per tile, stationary=S chunk, moving=ones[128,1]) accumulated in
#     one PSUM bank. psr is zeroed by an explicit DVE memset and the
#     ones-matmuls use start=False: per-column start=True would each mark
#     the whole 2KB PSUM zero-region, wiping sibling columns' sums.
#     The LAST head uses the ones path for ALL 16 tiles (plain column sums
#     need no symmetry) so no DVE reduction work sits near the drain.
# No collectives, no DRAM bounces, no cross-core latency chains.
#
# Engine budget/head (TimelineSim, ~17us period): ACT 16.8us (16 exps -
# the bottleneck), PE ~15.5 (MM1/MM2 13.6 + FC 1.7 + ones ~0), DVE ~15
# (7 reduces + 8 FC combines + outT copies + recip; the only engine
# besides ACT/PE that may touch PSUM - GPSIMD cannot), Pool idle.
# FC for head h runs during head h+1 (its reciprocal is ready at head h's
# end); the last head's FC drains through the freed psE banks.
# Loads are coarse (HWDGE costs ~0.6us per descriptor) with small/urgent
# tensors first; outputs stream per q-block as the drain completes.

import numpy as np
import ml_dtypes

N, S, D_IN, HEADS = 4, 2048, 512, 8
HEAD_DIM = 128          # 2*D_IN // HEADS
DF = 2 * D_IN           # 1024 combined features
QH = S // 2             # 1024 rows per core
NB = S // 128           # 16 l-blocks
QB = QH // 128          # 8 q-blocks per core
ISQ = 1.0 / float(np.sqrt(np.float32(HEAD_DIM)))

_CACHED_NC = None


def _build_nc(no_collective=False):
    # no_collective kept for API compat; this kernel has no collectives.
    import concourse.mybir as mybir
    import concourse.tile as tile
    from concourse import bacc
    from concourse.bass import ts

    f32 = mybir.dt.float32
    bf16 = mybir.dt.bfloat16
    Exp = mybir.ActivationFunctionType.Exp
    mult = mybir.AluOpType.mult
    add = mybir.AluOpType.add
    AxX = mybir.AxisListType.X

    nc = bacc.Bacc("TRN2", target_bir_lowering=False, debug=False, num_devices=8)

    xt = nc.dram_tensor("xt", [DF, S], bf16, kind="ExternalInput")      # X^T, l-permuted
    xtq = nc.dram_tensor("xtq", [DF, QH], bf16, kind="ExternalInput")   # X^T my-half cols
    x = nc.dram_tensor("x", [S, DF], bf16, kind="ExternalInput")        # X, l-permuted
    w = nc.dram_tensor("w", [DF, D_IN], bf16, kind="ExternalInput")     # W_out
    bias = nc.dram_tensor("bias", [128, D_IN], f32, kind="ExternalInput")
    ones = nc.dram_tensor("ones", [128, 1], bf16, kind="ExternalInput")
    out = nc.dram_tensor("out", [QH, D_IN], f32, kind="ExternalOutput")

    LAST = HEADS - 1

    with tile.TileContext(nc) as tc:
        with (
            tc.tile_pool(name="persist", bufs=1) as pers,
            tc.tile_pool(name="spool", bufs=8) as spool,
            tc.tile_pool(name="outp", bufs=3) as outp,
            tc.tile_pool(name="rpool", bufs=3) as rpool,
            tc.tile_pool(name="psE", bufs=2, space="PSUM") as psE,
            tc.tile_pool(name="psO", bufs=1, space="PSUM") as psO,
            tc.tile_pool(name="psFC", bufs=1, space="PSUM") as psFC,
            tc.tile_pool(name="psR", bufs=1, space="PSUM") as psR,
        ):
            # ---- persistent SBUF data ----
            xt_sb = pers.tile([128, HEADS, S], bf16, name="xt_sb")
            xtq_sb = pers.tile([128, HEADS, QH], bf16, name="xtq_sb")
            x_sb = pers.tile([128, NB, DF], bf16, name="x_sb")
            w_sb = pers.tile([128, HEADS, D_IN], bf16, name="w_sb")
            bias_sb = pers.tile([128, D_IN], f32, name="bias_sb")
            ones_sb = pers.tile([128, 1], bf16, name="ones_sb")

            xt_r = xt.ap().rearrange("(h p) s -> p h s", p=128)
            xtq_r = xtq.ap().rearrange("(h p) s -> p h s", p=128)
            x_r = x.ap().rearrange("(o p) f -> p o f", p=128)
            w_r = w.ap().rearrange("(h p) o -> p h o", p=128)

            # PE p-state warm-up: a ~0-cost matmul at t=0 starts the
            # ramp clock so the first real MM1 runs at full clock.
            cone = nc.const_aps.scalar_like(1.0, bias_sb[:, 0:1])
            pwarm = psFC.tile([128, D_IN], f32, tag="pfc", name="pwarm")
            nc.tensor.matmul(pwarm[0:1, 0:1], cone, cone, start=True, stop=True)

            # Startup loads. HWDGE costs ~0.6us per descriptor, so COUNT
            # matters: coarse chunks, few DMAs, small/urgent tensors first.
            nc.sync.dma_start(xtq_sb[:, 0, 0:512], xtq_r[:, 0, 0:512])
            nc.sync.dma_start(xt_sb[:, 0, 0:512], xt_r[:, 0, 0:512])
            nc.sync.dma_start(xtq_sb[:, 0, 512:1024], xtq_r[:, 0, 512:1024])
            nc.sync.dma_start(xt_sb[:, 0, 512:1024], xt_r[:, 0, 512:1024])
            nc.sync.dma_start(xt_sb[:, 0, 1024:2048], xt_r[:, 0, 1024:2048])
            nc.sync.dma_start(ones_sb[:], ones.ap())
            nc.sync.dma_start(bias_sb[:], bias.ap())
            for i in range(0, NB, 4):
                nc.sync.dma_start(x_sb[:, i : i + 4, :], x_r[:, i : i + 4, :])
            for h in range(1, HEADS):
                nc.sync.dma_start(xt_sb[:, h, :], xt_r[:, h, :])
                nc.sync.dma_start(xtq_sb[:, h, :], xtq_r[:, h, :])
            nc.sync.dma_start(w_sb[:], w_r[:])

            # fc accumulators, persist across heads
            accs = []
            for j in range(QB):
                a = pers.tile([128, D_IN], f32, name=f"acc{j}")
                accs.append(a)

            # fc_steps[h]: QB combine steps, popped during head h+1
            fc_steps = {}

            def make_fc_steps(h, outT, recip):
                steps = []
                ring = []
                for j in range(QB):
                    def step(h=h, j=j, outT=outT, recip=recip, eng=None,
                             ring=ring):
                        if h == LAST:
                            # 4-slot ring over the freed psE banks
                            if j % 2 == 0:
                                ring.append(psE.tile([128, QH], f32,
                                                     tag="pse", name="pfc"))
                            t = ring[(j // 2) % 2] if len(ring) >= 2 else ring[0]
                            pfc = t[:, (j % 2) * D_IN : (j % 2 + 1) * D_IN]
                        else:
                            pfc = psFC.tile([128, D_IN], f32, tag="pfc", name="pfc")
                        nc.tensor.matmul(pfc[:], outT[:, ts(j, 128)],
                                         w_sb[:, h, :], start=True, stop=True)
                        eng = nc.vector
                        if h == 0:
                            eng.scalar_tensor_tensor(
                                accs[j][:], pfc[:], recip[:, j : j + 1],
                                bias_sb[:], mult, add)
                        else:
                            eng.scalar_tensor_tensor(
                                accs[j][:], pfc[:], recip[:, j : j + 1],
                                accs[j][:], mult, add)
                        if h == LAST:
                            nc.sync.dma_start(out.ap()[ts(j, 128), :], accs[j][:])
                    steps.append(step)
                fc_steps[h] = steps

            pse_tiles = {}

            def mm1(h, i):
                pse = psE.tile([128, QH], f32, tag="pse", name="pse")
                lhs1 = xt_sb[:, h, ts(i, 128)]
                nc.tensor.matmul(pse[:, 0:512], lhs1, xtq_sb[:, h, 0:512],
                                 start=True, stop=True)
                nc.tensor.matmul(pse[:, 512:1024], lhs1,
                                 xtq_sb[:, h, 512:1024],
                                 start=True, stop=True)
                pse_tiles[(h, i)] = pse

            mm1(0, 0)
            mm1(0, 1)
            for h in range(HEADS):
                is_last = h == LAST
                racc = pers.tile([128, QB], f32, name=f"racc{h}")
                psr = psR.tile([128, QB], f32, tag="psr", name="psr")
                # zero explicitly: per-column start=True matmuls would each
                # mark the whole 2KB PSUM zero-region pending-zero, wiping
                # sibling columns' already-accumulated first terms
                nc.vector.memset(psr[:], 0.0)
                pso = psO.tile([128, QH], f32, tag="pso", name="pso")

                for i in range(NB):
                    pse = pse_tiles.pop((h, i))
                    s_i = spool.tile([128, QH], bf16, tag="s", name="s_i")
                    # own-half rowsums: DVE reduce, except the last own tile
                    # which uses ACT accum_out to keep DVE under its window.
                    # The last head uses the ones-matmul path for ALL tiles
                    # (direct column sums; no DVE work left near the drain).
                    use_acc = (i == QB - 1) and not is_last
                    nc.scalar.activation(
                        s_i[:], pse[:], Exp, bias=0.0, scale=ISQ,
                        accum_out=racc[:, i : i + 1] if use_acc else None)
                    if i + 2 < NB:
                        mm1(h, i + 2)
                    if i < QB - 1 and not is_last:
                        nc.vector.tensor_reduce(
                            racc[:, i : i + 1], s_i[:], AxX, add)
                    lhs2 = x_sb[:, i, ts(h, 128)]
                    nc.tensor.matmul(pso[:, 0:512], lhs2, s_i[:, 0:512],
                                     start=(i == 0), stop=(i == NB - 1))
                    nc.tensor.matmul(pso[:, 512:1024], lhs2, s_i[:, 512:1024],
                                     start=(i == 0), stop=(i == NB - 1))
                    if i >= QB or is_last:
                        # peer-half rowsums (all tiles for the last head):
                        # column sums via ones-matmuls accumulated in PSUM
                        for j in range(QB):
                            nc.tensor.matmul(
                                psr[:, j : j + 1], s_i[:, ts(j, 128)],
                                ones_sb[:], start=False,
                                stop=(i == NB - 1), skip_group_check=True)
                    # fc work of head h-1, one step per iteration in the
                    # later iterations where DVE has slack (ending 2 before
                    # the head boundary so no combine lingers into the drain)
                    if h >= 1 and QB - 4 <= i and i - (QB - 4) < len(fc_steps[h - 1]):
                        fc_steps[h - 1][i - (QB - 4)]()
                    # cross-head MM1 prefetch at iteration end: by now the
                    # target psE buffer's reader (exp of tile i-1) is done,
                    # so this does not park PE's in-order queue
                    if i + 2 >= NB and h + 1 < HEADS:
                        mm1(h + 1, i + 2 - NB)

                # complete rowsums + reciprocal, all local, at head end
                recip = rpool.tile([128, QB], f32, tag="recip", name="recip")
                if is_last:
                    nc.vector.reciprocal(recip[:], psr[:])
                else:
                    rsum = rpool.tile([128, QB], f32, tag="rsum", name="rsum")
                    nc.vector.tensor_tensor(rsum[:], racc[:], psr[:], add)
                    nc.vector.reciprocal(recip[:], rsum[:])

                # outT copy; the last head's runs on ACT (idle after its
                # final exp, and ACT may read PSUM) so the drain isn't
                # serialized behind DVE's queue
                outT = outp.tile([128, QH], bf16, tag="outT", name="outT")
                if is_last:
                    Copy = mybir.ActivationFunctionType.Copy
                    nc.scalar.activation(outT[:, 0:512], pso[:, 0:512], Copy)
                    nc.scalar.activation(outT[:, 512:1024], pso[:, 512:1024], Copy)
                else:
                    nc.vector.tensor_copy(outT[:, 0:512], pso[:, 0:512])
                    nc.vector.tensor_copy(outT[:, 512:1024], pso[:, 512:1024])

                make_fc_steps(h, outT, recip)

            # drain the last head's fc
            for step in fc_steps[LAST]:
                step()

    nc.compile()
    return nc


def _get_nc():
    global _CACHED_NC
    if _CACHED_NC is None:
        _CACHED_NC = _build_nc()
    return _CACHED_NC


def _in_maps(output1, output2, W_out, b_out):
    bf = ml_dtypes.bfloat16
    X = np.concatenate([np.asarray(output1), np.asarray(output2)], axis=2)
    Xb = X.astype(bf)
    Wb = np.ascontiguousarray(np.asarray(W_out).astype(bf))
    bias_full = np.ascontiguousarray(
        np.broadcast_to(np.asarray(b_out).astype(np.float32), (128, D_IN)))
    ones_t = np.ones((128, 1), bf)

    in_maps = []
    for c in range(8):
        n, half = c // 2, c % 2
        Xn = np.asarray(Xb[n])                      # [S, DF]
        if half == 1:
            # key reorder: my q-half rows first (identity for even cores)
            Xn = np.concatenate([Xn[QH:], Xn[:QH]], axis=0)
        Xn = np.ascontiguousarray(Xn)
        XTn = np.ascontiguousarray(Xn.T)            # [DF, S] (l-permuted)
        # my q columns: after the permutation they are always cols 0..QH
        in_maps.append({
            "x": Xn,
            "xt": XTn,
            "xtq": np.ascontiguousarray(XTn[:, 0:QH]),
            "w": Wb,
            "bias": bias_full,
            "ones": ones_t,
        })
    return in_maps


def kernel(output1, output2, W_out, b_out):
    from concourse.bass_utils import run_bass_kernel_spmd

    in_maps = _in_maps(output1, output2, W_out, b_out)
    nc = _get_nc()
    res = run_bass_kernel_spmd(nc, in_maps, core_ids=list(range(8)))

    full = np.empty((N, S, D_IN), np.float32)
    for c in range(8):
        n, half = c // 2, c % 2
        full[n, half * QH : (half + 1) * QH, :] = res.results[c]["out"]
    return full


# revision 24
# speedup vs baseline: 4.8997x; 4.8997x over previous
# Trainium2 Bass kernel for nn_AttentionCombiner — collective-free.
#
# Sharding: 8 cores = 4 batches x 2 q-halves (all 8 heads per core).
# The host reorders the keys per core so l-blocks 0..7 are the core's OWN
# q-rows and 8..15 the peer's (identity for even cores). Softmax row-sums
# are then computed fully locally:
#   - own half  (Sum_{l in my half} S[l,q] = Sum_{q'} S[q,q'] by symmetry):
#     DVE tensor_reduce over the exp'd S tiles;
#   - peer half: column sums of S tiles 8..15 via ap_size-1 ones-matmuls on
#     PE (8 per tile, stationary=S chunk, moving=ones[128,1]) accumulated in
#     one PSUM bank. psr is zeroed by an explicit DVE memset and the
#     ones-matmuls use start=False: per-column start=True would each mark
#     the whole 2KB PSUM zero-region, wiping sibling columns' sums.
#     The LAST head uses the ones path for ALL 16 tiles (plain column sums
#     need no symmetry) so no DVE reduction work sits near the drain.
# No collectives, no DRAM bounces, no cross-core latency chains.
#
# Engine budget/head (TimelineSim, ~16.6us period): ACT 16.6us (16 exps -
# the bottleneck), PE ~15.3 (MM1/MM2 13.6 + FC 1.7 + ones ~0), DVE ~15.8
# (8 reduces + 8 FC combines + outT copies + recip; the only engine
# besides ACT/PE that may touch PSUM - GPSIMD cannot), Pool idle.
# FC for head h runs during head h+1 (its reciprocal is ready at head h's
# end); the last head's FC drains through the freed psE banks.
# Loads are coarse (HWDGE costs ~0.6us per descriptor) with small/urgent
# tensors first; outputs stream per q-block as the drain completes.

import numpy as np
import ml_dtypes

N, S, D_IN, HEADS = 4, 2048, 512, 8
HEAD_DIM = 128          # 2*D_IN // HEADS
DF = 2 * D_IN           # 1024 combined features
QH = S // 2             # 1024 rows per core
NB = S // 128           # 16 l-blocks
QB = QH // 128          # 8 q-blocks per core
ISQ = 1.0 / float(np.sqrt(np.float32(HEAD_DIM)))

_CACHED_NC = None


def _build_nc(no_collective=False):
    # no_collective kept for API compat; this kernel has no collectives.
    import concourse.mybir as mybir
    import concourse.tile as tile
    from concourse import bacc
    from concourse.bass import ts

    f32 = mybir.dt.float32
    bf16 = mybir.dt.bfloat16
    Exp = mybir.ActivationFunctionType.Exp
    mult = mybir.AluOpType.mult
    add = mybir.AluOpType.add
    AxX = mybir.AxisListType.X

    nc = bacc.Bacc("TRN2", target_bir_lowering=False, debug=False, num_devices=8)

    xt = nc.dram_tensor("xt", [DF, S], bf16, kind="ExternalInput")      # X^T, l-permuted
    xtq = nc.dram_tensor("xtq", [DF, QH], bf16, kind="ExternalInput")   # X^T my-half cols
    x = nc.dram_tensor("x", [S, DF], bf16, kind="ExternalInput")        # X, l-permuted
    w = nc.dram_tensor("w", [DF, D_IN], bf16, kind="ExternalInput")     # W_out
    bias = nc.dram_tensor("bias", [128, D_IN], f32, kind="ExternalInput")
    ones = nc.dram_tensor("ones", [128, 1], bf16, kind="ExternalInput")
    out = nc.dram_tensor("out", [QH, D_IN], f32, kind="ExternalOutput")

    LAST = HEADS - 1

    with tile.TileContext(nc) as tc:
        with (
            tc.tile_pool(name="persist", bufs=1) as pers,
            tc.tile_pool(name="spool", bufs=12) as spool,
            tc.tile_pool(name="outp", bufs=3) as outp,
            tc.tile_pool(name="rpool", bufs=3) as rpool,
            tc.tile_pool(name="psE", bufs=2, space="PSUM") as psE,
            tc.tile_pool(name="psO", bufs=1, space="PSUM") as psO,
            tc.tile_pool(name="psFC", bufs=1, space="PSUM") as psFC,
            tc.tile_pool(name="psR", bufs=1, space="PSUM") as psR,
        ):
            # ---- persistent SBUF data ----
            xt_sb = pers.tile([128, HEADS, S], bf16, name="xt_sb")
            xtq_sb = pers.tile([128, HEADS, QH], bf16, name="xtq_sb")
            x_sb = pers.tile([128, NB, DF], bf16, name="x_sb")
            w_sb = pers.tile([128, HEADS, D_IN], bf16, name="w_sb")
            bias_sb = pers.tile([128, D_IN], f32, name="bias_sb")
            ones_sb = pers.tile([128, 1], bf16, name="ones_sb")

            xt_r = xt.ap().rearrange("(h p) s -> p h s", p=128)
            xtq_r = xtq.ap().rearrange("(h p) s -> p h s", p=128)
            x_r = x.ap().rearrange("(o p) f -> p o f", p=128)
            w_r = w.ap().rearrange("(h p) o -> p h o", p=128)

            # PE p-state warm-up: a ~0-cost matmul at t=0 starts the
            # ramp clock so the first real MM1 runs at full clock.
            cone = nc.const_aps.scalar_like(1.0, bias_sb[:, 0:1])
            pwarm = psFC.tile([128, D_IN], f32, tag="pfc", name="pwarm")
            nc.tensor.matmul(pwarm[0:1, 0:1], cone, cone, start=True, stop=True)

            # Startup loads. HWDGE costs ~0.6us per descriptor AND the
            # data channel serializes, so order = exactly when each tensor
            # is first consumed: MM1(0,0) needs xtq h0 + xt tile 0;
            # MM2(0,i) needs x tile i at ~1us intervals; later heads and
            # fc-only tensors (bias, w) come last.
            nc.sync.dma_start(xt_sb[:, 0, 0:128], xt_r[:, 0, 0:128])
            nc.sync.dma_start(xtq_sb[:, 0, :], xtq_r[:, 0, :])
            nc.sync.dma_start(xt_sb[:, 0, 128:512], xt_r[:, 0, 128:512])
            nc.sync.dma_start(x_sb[:, 0:1, :], x_r[:, 0:1, :])
            nc.sync.dma_start(xt_sb[:, 0, 512:2048], xt_r[:, 0, 512:2048])
            nc.sync.dma_start(x_sb[:, 1:2, :], x_r[:, 1:2, :])
            nc.sync.dma_start(x_sb[:, 2:4, :], x_r[:, 2:4, :])
            nc.sync.dma_start(ones_sb[:], ones.ap())
            nc.sync.dma_start(x_sb[:, 4:8, :], x_r[:, 4:8, :])
            nc.sync.dma_start(xt_sb[:, 1, :], xt_r[:, 1, :])
            nc.sync.dma_start(xtq_sb[:, 1, :], xtq_r[:, 1, :])
            nc.sync.dma_start(bias_sb[:], bias.ap())
            nc.sync.dma_start(x_sb[:, 8:12, :], x_r[:, 8:12, :])
            nc.sync.dma_start(x_sb[:, 12:16, :], x_r[:, 12:16, :])
            for h in range(2, HEADS):
                nc.sync.dma_start(xt_sb[:, h, :], xt_r[:, h, :])
                nc.sync.dma_start(xtq_sb[:, h, :], xtq_r[:, h, :])
            nc.sync.dma_start(w_sb[:], w_r[:])

            # fc accumulators, persist across heads
            accs = []
            for j in range(QB):
                a = pers.tile([128, D_IN], f32, name=f"acc{j}")
                accs.append(a)

            # fc_steps[h]: QB combine steps, popped during head h+1
            fc_steps = {}

            def make_fc_steps(h, outT, recip):
                steps = []
                ring = []
                for j in range(QB):
                    def step(h=h, j=j, outT=outT, recip=recip, eng=None,
                             ring=ring):
                        if h == LAST:
                            # 4-slot ring over the freed psE banks
                            if j % 2 == 0:
                                ring.append(psE.tile([128, QH], f32,
                                                     tag="pse", name="pfc"))
                            t = ring[(j // 2) % 2] if len(ring) >= 2 else ring[0]
                            pfc = t[:, (j % 2) * D_IN : (j % 2 + 1) * D_IN]
                        else:
                            pfc = psFC.tile([128, D_IN], f32, tag="pfc", name="pfc")
                        nc.tensor.matmul(pfc[:], outT[:, ts(j, 128)],
                                         w_sb[:, h, :], start=True, stop=True)
                        eng = nc.vector
                        if h == 0:
                            eng.scalar_tensor_tensor(
                                accs[j][:], pfc[:], recip[:, j : j + 1],
                                bias_sb[:], mult, add)
                        else:
                            eng.scalar_tensor_tensor(
                                accs[j][:], pfc[:], recip[:, j : j + 1],
                                accs[j][:], mult, add)
                        if h == LAST:
                            nc.sync.dma_start(out.ap()[ts(j, 128), :], accs[j][:])
                    steps.append(step)
                fc_steps[h] = steps

            pse_tiles = {}

            def mm1(h, i):
                pse = psE.tile([128, QH], f32, tag="pse", name="pse")
                lhs1 = xt_sb[:, h, ts(i, 128)]
                nc.tensor.matmul(pse[:, 0:512], lhs1, xtq_sb[:, h, 0:512],
                                 start=True, stop=True)
                nc.tensor.matmul(pse[:, 512:1024], lhs1,
                                 xtq_sb[:, h, 512:1024],
                                 start=True, stop=True)
                pse_tiles[(h, i)] = pse

            mm1(0, 0)
            mm1(0, 1)
            for h in range(HEADS):
                is_last = h == LAST
                racc = pers.tile([128, QB], f32, name=f"racc{h}")
                psr = psR.tile([128, QB], f32, tag="psr", name="psr")
                # zero explicitly: per-column start=True matmuls would each
                # mark the whole 2KB PSUM zero-region pending-zero, wiping
                # sibling columns' already-accumulated first terms
                nc.vector.memset(psr[:], 0.0)
                pso = psO.tile([128, QH], f32, tag="pso", name="pso")

                s_tiles = {}
                for i in range(NB):
                    pse = pse_tiles.pop((h, i))
                    s_i = spool.tile([128, QH], bf16, tag="s", name="s_i")
                    s_tiles[i] = s_i
                    # own-half rowsums: DVE reduce. The last head uses
                    # the ones-matmul path for ALL tiles instead (direct
                    # column sums; no DVE work left near the drain).
                    use_acc = False
                    nc.scalar.activation(
                        s_i[:], pse[:], Exp, bias=0.0, scale=ISQ,
                        accum_out=racc[:, i : i + 1] if use_acc else None)
                    if i + 2 < NB:
                        mm1(h, i + 2)
                    # reduces for the first QB-2 tiles run inline; the
                    # last two own-half reduces are deferred two iterations
                    # so DVE's first half-head (8x1.127us) does not exceed
                    # the 8.3us ACT half-period and lag s-tile releases
                    if i < QB - 2 and not is_last:
                        nc.vector.tensor_reduce(
                            racc[:, i : i + 1], s_i[:], AxX, add)
                    if QB <= i < QB + 2 and not is_last:
                        i_d = i - 2
                        nc.vector.tensor_reduce(
                            racc[:, i_d : i_d + 1], s_tiles[i_d][:], AxX, add)
                    lhs2 = x_sb[:, i, ts(h, 128)]
                    nc.tensor.matmul(pso[:, 0:512], lhs2, s_i[:, 0:512],
                                     start=(i == 0), stop=(i == NB - 1))
                    nc.tensor.matmul(pso[:, 512:1024], lhs2, s_i[:, 512:1024],
                                     start=(i == 0), stop=(i == NB - 1))
                    # cross-head MM1 prefetch right after MM2, ahead of the
                    # ones batch whose SEQ dispatches would otherwise delay
                    # the next head's second exp
                    if i + 2 >= NB and h + 1 < HEADS:
                        mm1(h + 1, i + 2 - NB)
                    if i >= QB or is_last:
                        # peer-half rowsums (all tiles for the last head):
                        # column sums via ones-matmuls accumulated in PSUM
                        for j in range(QB):
                            nc.tensor.matmul(
                                psr[:, j : j + 1], s_i[:, ts(j, 128)],
                                ones_sb[:], start=False,
                                stop=(i == NB - 1), skip_group_check=True)
                    # fc work of head h-1, one step every OTHER iteration:
                    # a step per iteration pushes PE 27ns past the ACT period
                    # for 8 straight iterations, building a backlog the psE
                    # prefetch can't absorb (recip h-1 is ready at the head
                    # boundary, so spreading across the whole head is safe)
                    if h >= 1 and i % 2 == 0 and i // 2 < len(fc_steps[h - 1]):
                        fc_steps[h - 1][i // 2]()

                # complete rowsums + reciprocal, all local, at head end
                recip = rpool.tile([128, QB], f32, tag="recip", name="recip")
                if is_last:
                    nc.vector.reciprocal(recip[:], psr[:])
                else:
                    rsum = rpool.tile([128, QB], f32, tag="rsum", name="rsum")
                    nc.vector.tensor_tensor(rsum[:], racc[:], psr[:], add)
                    nc.vector.reciprocal(recip[:], rsum[:])

                # outT copy; the last head's runs on ACT (idle after its
                # final exp, and ACT may read PSUM) so the drain isn't
                # serialized behind DVE's queue
                outT = outp.tile([128, QH], bf16, tag="outT", name="outT")
                if is_last:
                    Copy = mybir.ActivationFunctionType.Copy
                    for c0 in range(0, QH, 256):
                        nc.scalar.activation(
                            outT[:, c0 : c0 + 256], pso[:, c0 : c0 + 256], Copy)
                else:
                    nc.vector.tensor_copy(outT[:, 0:512], pso[:, 0:512])
                    nc.vector.tensor_copy(outT[:, 512:1024], pso[:, 512:1024])

                make_fc_steps(h, outT, recip)

            # drain the last head's fc
            for step in fc_steps[LAST]:
                step()

    nc.compile()
    return nc


def _get_nc():
    global _CACHED_NC
    if _CACHED_NC is None:
        _CACHED_NC = _build_nc()
    return _CACHED_NC


def _in_maps(output1, output2, W_out, b_out):
    bf = ml_dtypes.bfloat16
    X = np.concatenate([np.asarray(output1), np.asarray(output2)], axis=2)
    Xb = X.astype(bf)
    Wb = np.ascontiguousarray(np.asarray(W_out).astype(bf))
    bias_full = np.ascontiguousarray(
        np.broadcast_to(np.asarray(b_out).astype(np.float32), (128, D_IN)))
    ones_t = np.ones((128, 1), bf)

    in_maps = []
    for c in range(8):
        n, half = c // 2, c % 2
        Xn = np.asarray(Xb[n])                      # [S, DF]
        if half == 1:
            # key reorder: my q-half rows first (identity for even cores)
            Xn = np.concatenate([Xn[QH:], Xn[:QH]], axis=0)
        Xn = np.ascontiguousarray(Xn)
        XTn = np.ascontiguousarray(Xn.T)            # [DF, S] (l-permuted)
        # my q columns: after the permutation they are always cols 0..QH
        in_maps.append({
            "x": Xn,
            "xt": XTn,
            "xtq": np.ascontiguousarray(XTn[:, 0:QH]),
            "w": Wb,
            "bias": bias_full,
            "ones": ones_t,
        })
    return in_maps


def kernel(output1, output2, W_out, b_out):
    from concourse.bass_utils import run_bass_kernel_spmd

    in_maps = _in_maps(output1, output2, W_out, b_out)
    nc = _get_nc()
    res = run_bass_kernel_spmd(nc, in_maps, core_ids=list(range(8)))

    full = np.empty((N, S, D_IN), np.float32)
    for c in range(8):
        n, half = c // 2, c % 2
        full[n, half * QH : (half + 1) * QH, :] = res.results[c]["out"]
    return full
